# revision 16
# baseline (speedup 1.0000x reference)
"""Trainium2 Bass kernel for vq_codebook argmin (nn_GUMSampler) — v3.

Per pixel p (4M pixels): d2[v] = ||z_p - vertex_v||^2 over 16 vertices in
R^15; outputs argmin index (int32) and min distance (f32).

Pixels sharded 8 ways across cores; per core N=524288 pixels in G=8 groups.

Per-core pipeline (64 rounds of 8192 pixels, FD=1024 per group-round):
  PE    d2 = w1 @ z + wsq @ z^2 into PSUM rows 8v+g   (fp16 in, f32 accum;
        vv split hi/lo over the two ones-row slots for near-f32 accuracy)
  DVE+ACT  z^2 in fp16 (free-dim split across both engines)
  ACT   evacuate PSUM -> SBUF fused with sqrt: ev = sqrt(d2) (monotonic,
        so min/argmin are preserved and no separate sqrt pass is needed)
  DVE   pack candidate index v into the low 4 mantissa bits, touching only
        the LOW uint16 of each f32 (TS 2x_2P on half the elements)
  PE    transpose packed f32 (bit-exact) -> PSUM pixel-major
  DVE   one segmented strided tensor_reduce(min) over the 16 candidates
        (reads PSUM directly; packed min = value + argmin in one word)
The packed f32 min IS the output word: its value is dmin (sqrt is fused
into the PSUM evacuation, low-bit perturbation ~1e-6) and its low 4 bits
are the argmin index, extracted host-side. One output DMA at the end.
"""

import sys

sys.path.insert(0, "/opt/trn_rl_repo")

from contextlib import ExitStack

import numpy as np

import concourse.bacc as bacc
import concourse.tile as tile
from concourse import mybir
from concourse.bass_utils import run_bass_kernel_spmd

F32 = mybir.dt.float32
F16 = mybir.dt.float16
BF16 = mybir.dt.bfloat16
I32 = mybir.dt.int32
I8 = mybir.dt.int8
U16 = mybir.dt.uint16
MIN = mybir.AluOpType.min

K = 16            # vertices
C = 15            # channels
G = 8             # pixel groups (PSUM row = 8v+g)
RFD = 1024        # pixels per group per round
N_CORES = 8
LX = LY = 2048
N_TOTAL = LX * LY
N_LOC = N_TOTAL // N_CORES          # 524288
N_ROUNDS = N_LOC // (G * RFD)       # 64
GBLK = N_LOC // G                   # 65536 pixels per group
ZSQ_DVE = 320                       # zsq slab columns done on DVE (rest on ACT)
OPR = RFD // 16                     # output columns per round

_CACHE = {}


def build_nc(n_rounds=N_ROUNDS):
    gblk = n_rounds * RFD
    nc = bacc.Bacc("TRN2", target_bir_lowering=False, debug=False)

    zt_d = nc.dram_tensor("zt", [128, gblk], F16, kind="ExternalInput")
    w1_d = nc.dram_tensor("w1", [128, 128], F16, kind="ExternalInput")
    wsq_d = nc.dram_tensor("wsq", [128, 128], F16, kind="ExternalInput")
    ident_d = nc.dram_tensor("ident", [128, 128], F32, kind="ExternalInput")
    vvec_d = nc.dram_tensor("vvec", [128, 1], U16, kind="ExternalInput")
    dm_d = nc.dram_tensor("dm_o", [128, n_rounds * OPR], F32, kind="ExternalOutput")

    with tile.TileContext(nc) as tc, ExitStack() as ctx:
        cpool = ctx.enter_context(tc.tile_pool(name="consts", bufs=1))
        w1_s = cpool.tile([128, 128], F16)
        wsq_s = cpool.tile([128, 128], F16)
        ident_s = cpool.tile([128, 128], F32)
        vvec_s = cpool.tile([128, 1], U16)
        nc.sync.dma_start(w1_s[:], w1_d[:])
        nc.sync.dma_start(wsq_s[:], wsq_d[:])
        nc.sync.dma_start(ident_s[:], ident_d[:])
        nc.sync.dma_start(vvec_s[:], vvec_d[:])

        ztpool = ctx.enter_context(tc.tile_pool(name="zt", bufs=1))
        zt = ztpool.tile([128, gblk], F16)
        # load z in graded chunks (small first) so compute starts early
        if n_rounds >= 16:
            bounds = [0, 1, 3, 6, 14]
            step = (n_rounds - 14) // 5
            for i in range(1, 5):
                bounds.append(14 + step * i)
            bounds.append(n_rounds)
        else:
            bounds = list(range(n_rounds + 1))
        for ch in range(len(bounds) - 1):
            lo, hi = bounds[ch] * RFD, bounds[ch + 1] * RFD
            if hi > lo:
                nc.sync.dma_start(zt[:, lo:hi], zt_d[:, lo:hi])

        zsqpool = ctx.enter_context(tc.tile_pool(name="zsq", bufs=3))
        pspool = ctx.enter_context(tc.tile_pool(name="d2ps", bufs=2, space="PSUM"))
        epool = ctx.enter_context(tc.tile_pool(name="evac", bufs=4))
        tpool = ctx.enter_context(tc.tile_pool(name="tps", bufs=2, space="PSUM"))
        pmpool = ctx.enter_context(tc.tile_pool(name="pm", bufs=4))
        opool = ctx.enter_context(tc.tile_pool(name="outs", bufs=1))
        dm_acc = opool.tile([128, n_rounds * OPR], F32)

        SLAB = 2 * RFD
        for s in range(n_rounds // 2):
            zslab = zt[:, s * SLAB : (s + 1) * SLAB]

            # ---- z^2 fp16 for the whole slab, split across DVE and ACT ----
            zsq = zsqpool.tile([128, SLAB], F16)
            nc.vector.tensor_tensor(
                zsq[:, :ZSQ_DVE], zslab[:, :ZSQ_DVE], zslab[:, :ZSQ_DVE],
                mybir.AluOpType.mult,
            )
            nc.scalar.square(zsq[:, ZSQ_DVE:], zslab[:, ZSQ_DVE:])

            ev = epool.tile([128, SLAB], F32)
            for half in range(2):
                r = 2 * s + half
                zs = zslab[:, half * RFD : (half + 1) * RFD]
                evh = ev[:, half * RFD : (half + 1) * RFD]

                # ---- d2 into PSUM: rows 8v+g ----
                ps = pspool.tile([128, RFD], F32)
                for h in range(RFD // 512):
                    sl = slice(512 * h, 512 * h + 512)
                    nc.tensor.matmul(ps[:, sl], w1_s[:], zs[:, sl], start=True, stop=False)
                for h in range(RFD // 512):
                    sl = slice(512 * h, 512 * h + 512)
                    nc.tensor.matmul(
                        ps[:, sl], wsq_s[:], zsq[:, half * RFD + sl.start : half * RFD + sl.stop],
                        start=False, stop=True,
                    )

                # ---- evacuate to SBUF fused with sqrt: ev = sqrt(d2) ----
                nc.scalar.sqrt(evh, ps[:])

            # ---- pack index into low 4 mantissa bits, whole slab at once ----
            lo = ev[:].bitcast(U16).rearrange("p (n two) -> p n two", two=2)[:, :, 0]
            nc.vector.tensor_scalar(
                lo, lo, -16, vvec_s[:],
                op0=mybir.AluOpType.bitwise_and, op1=mybir.AluOpType.bitwise_or,
            )

            for half in range(2):
                r = 2 * s + half
                evh = ev[:, half * RFD : (half + 1) * RFD]

                # ---- transpose to pixel-major PSUM (bit-exact f32) ----
                tp = tpool.tile([128, RFD], F32)
                for b in range(RFD // 128):
                    sl = slice(128 * b, 128 * b + 128)
                    nc.tensor.transpose(tp[:, sl], evh[:, sl], ident_s[:])

                # ---- segmented strided min-reduce -> packed dmin ----
                osl = slice(r * OPR, r * OPR + OPR)
                vview = tp[:].rearrange("p (b v g) -> p b g v", b=RFD // 128, v=K, g=G)
                nc.vector.tensor_reduce(dm_acc[:, osl], vview, mybir.AxisListType.X, MIN)

            # stream out finished output in two late waves so only the last
            # couple of rounds' sliver remains after the final reduce
            if n_rounds >= 16 and r == n_rounds - 1 - n_rounds // 8:
                cut1 = (r + 1) * OPR
                nc.sync.dma_start(dm_d[:, :cut1], dm_acc[:, :cut1])
            if n_rounds >= 16 and r == n_rounds - 3:
                cut2 = (r + 1) * OPR
                nc.sync.dma_start(dm_d[:, cut1:cut2], dm_acc[:, cut1:cut2])
        mid_out = (n_rounds - 2) * OPR if n_rounds >= 16 else 0
        nc.sync.dma_start(dm_d[:, mid_out:], dm_acc[:, mid_out:])

    nc.compile()
    return nc


def _weights(vertices):
    V = np.asarray(vertices, dtype=np.float32)              # (16, 15)
    vv = (V.astype(np.float64) ** 2).sum(1).astype(np.float32)
    vv_hi = vv.astype(np.float16)
    vv_lo = (vv - vv_hi.astype(np.float32)).astype(np.float16)
    w1 = np.zeros((128, 128), dtype=np.float16)
    wsq = np.zeros((128, 128), dtype=np.float16)
    for g in range(G):
        for v in range(K):
            col = 8 * v + g
            w1[16 * g : 16 * g + 15, col] = (-2.0 * V[v]).astype(np.float16)
            w1[16 * g + 15, col] = vv_hi[v]
            wsq[16 * g : 16 * g + 15, col] = 1.0
            wsq[16 * g + 15, col] = vv_lo[v]
    ident = np.eye(128, dtype=np.float32)
    vvec = (np.arange(128, dtype=np.uint16) >> 3).reshape(128, 1).astype(np.uint16)
    return w1, wsq, ident, vvec


def _prep_zt(z_core):
    # z_core: (15, N_loc) f32 -> [128, GBLK] f16 with ones rows at 16g+15
    n = z_core.shape[1]
    gblk = n // G
    zt = np.empty((128, gblk), dtype=np.float16)
    zg = z_core.reshape(C, G, gblk)
    for g in range(G):
        zt[16 * g : 16 * g + 15] = zg[:, g].astype(np.float16)
        zt[16 * g + 15] = np.float16(1.0)
    return zt


def _unscramble(arr, n_rounds):
    # arr [128, n_rounds*64] laid (q, (r, b, g)) -> per-group-major flat pixels
    a = np.asarray(arr).reshape(128, n_rounds, -1, 8)      # q, r, b, g
    a = a.transpose(3, 1, 2, 0)                            # g, r, b, q
    return np.ascontiguousarray(a).reshape(-1)             # g-major flat


def kernel(z, vertices):
    z = np.ascontiguousarray(np.asarray(z, dtype=np.float32))
    lx, ly = z.shape[1], z.shape[2]
    n = lx * ly
    z_fl = z.reshape(C, n)
    n_loc = n // N_CORES

    if "nc" not in _CACHE:
        _CACHE["nc"] = build_nc()
    nc = _CACHE["nc"]

    w1, wsq, ident, vvec = _weights(vertices)
    in_maps = []
    for c in range(N_CORES):
        zt = _prep_zt(z_fl[:, c * n_loc : (c + 1) * n_loc])
        in_maps.append(
            {"zt": zt, "w1": w1, "wsq": wsq, "ident": ident, "vvec": vvec}
        )
    res = run_bass_kernel_spmd(nc, in_maps, list(range(N_CORES)))

    Xs, Ds = [], []
    for c in range(N_CORES):
        dmc = _unscramble(np.asarray(res.results[c]["dm_o"]), N_ROUNDS)
        Xs.append((dmc.view(np.int32) & 15).astype(np.int32))
        Ds.append(dmc.astype(np.float32))
    X = np.concatenate(Xs).reshape(lx, ly)
    D = np.concatenate(Ds).reshape(lx, ly)
    return X, D


if __name__ == "__main__":
    print("module ok")


# revision 17
# speedup vs baseline: 1.0577x; 1.0577x over previous
"""Trainium2 Bass kernel for vq_codebook argmin (nn_GUMSampler) — v3.

Per pixel p (4M pixels): d2[v] = ||z_p - vertex_v||^2 over 16 vertices in
R^15; outputs argmin index (int32) and min distance (f32).

Pixels sharded 8 ways across cores; per core N=524288 pixels in G=8 groups.

Per-core pipeline (64 rounds of 8192 pixels, FD=1024 per group-round):
  PE    d2 = w1 @ z + wsq @ z^2 into PSUM rows 8v+g   (fp16 in, f32 accum;
        vv split hi/lo over the two ones-row slots for near-f32 accuracy)
  DVE+ACT  z^2 in fp16 (free-dim split across both engines)
  ACT   evacuate PSUM -> SBUF fused with sqrt: ev = sqrt(d2) (monotonic,
        so min/argmin are preserved and no separate sqrt pass is needed)
  DVE   pack candidate index v into the low 4 mantissa bits, touching only
        the LOW uint16 of each f32 (TS 2x_2P on half the elements)
  PE    transpose packed f32 (bit-exact) -> PSUM pixel-major
  DVE   one segmented strided tensor_reduce(min) over the 16 candidates
        (reads PSUM directly; packed min = value + argmin in one word)
The packed f32 min IS the output word: its value is dmin (sqrt is fused
into the PSUM evacuation, low-bit perturbation ~1e-6) and its low 4 bits
are the argmin index, extracted host-side. One output DMA at the end.
"""

import sys

sys.path.insert(0, "/opt/trn_rl_repo")

from contextlib import ExitStack

import numpy as np

import concourse.bacc as bacc
import concourse.tile as tile
from concourse import mybir
from concourse.bass_utils import run_bass_kernel_spmd

F32 = mybir.dt.float32
F16 = mybir.dt.float16
BF16 = mybir.dt.bfloat16
I32 = mybir.dt.int32
I8 = mybir.dt.int8
U16 = mybir.dt.uint16
MIN = mybir.AluOpType.min

K = 16            # vertices
C = 15            # channels
G = 8             # pixel groups (PSUM row = 8v+g)
RFD = 1024        # pixels per group per round
N_CORES = 8
LX = LY = 2048
N_TOTAL = LX * LY
N_LOC = N_TOTAL // N_CORES          # 524288
N_ROUNDS = N_LOC // (G * RFD)       # 64
GBLK = N_LOC // G                   # 65536 pixels per group
ZSQ_ACT = 768                       # zsq slab columns on ACT (rest on GPSIMD)
OPR = RFD // 16                     # output columns per round

_CACHE = {}


def build_nc(n_rounds=N_ROUNDS):
    gblk = n_rounds * RFD
    nc = bacc.Bacc("TRN2", target_bir_lowering=False, debug=False)

    zt_d = nc.dram_tensor("zt", [128, gblk], F16, kind="ExternalInput")
    w1_d = nc.dram_tensor("w1", [128, 128], F16, kind="ExternalInput")
    wsq_d = nc.dram_tensor("wsq", [128, 128], F16, kind="ExternalInput")
    ident_d = nc.dram_tensor("ident", [128, 128], F32, kind="ExternalInput")
    vvec_d = nc.dram_tensor("vvec", [128, 1], U16, kind="ExternalInput")
    dm_d = nc.dram_tensor("dm_o", [128, n_rounds * OPR], F32, kind="ExternalOutput")

    with tile.TileContext(nc) as tc, ExitStack() as ctx:
        cpool = ctx.enter_context(tc.tile_pool(name="consts", bufs=1))
        w1_s = cpool.tile([128, 128], F16)
        wsq_s = cpool.tile([128, 128], F16)
        ident_s = cpool.tile([128, 128], F32)
        vvec_s = cpool.tile([128, 1], U16)
        nc.sync.dma_start(w1_s[:], w1_d[:])
        nc.sync.dma_start(wsq_s[:], wsq_d[:])
        nc.sync.dma_start(ident_s[:], ident_d[:])
        nc.sync.dma_start(vvec_s[:], vvec_d[:])

        ztpool = ctx.enter_context(tc.tile_pool(name="zt", bufs=1))
        zt = ztpool.tile([128, gblk], F16)
        # load z in graded chunks (small first) so compute starts early
        if n_rounds >= 16:
            bounds = [0, 1, 3, 6, 14]
            step = (n_rounds - 14) // 5
            for i in range(1, 5):
                bounds.append(14 + step * i)
            bounds.append(n_rounds)
        else:
            bounds = list(range(n_rounds + 1))
        for ch in range(len(bounds) - 1):
            lo, hi = bounds[ch] * RFD, bounds[ch + 1] * RFD
            if hi > lo:
                nc.sync.dma_start(zt[:, lo:hi], zt_d[:, lo:hi])

        zsqpool = ctx.enter_context(tc.tile_pool(name="zsq", bufs=3))
        pspool = ctx.enter_context(tc.tile_pool(name="d2ps", bufs=2, space="PSUM"))
        epool = ctx.enter_context(tc.tile_pool(name="evac", bufs=4))
        tpool = ctx.enter_context(tc.tile_pool(name="tps", bufs=2, space="PSUM"))
        pmpool = ctx.enter_context(tc.tile_pool(name="pm", bufs=4))
        opool = ctx.enter_context(tc.tile_pool(name="outs", bufs=1))
        dm_acc = opool.tile([128, n_rounds * OPR], F32)

        SLAB = 2 * RFD
        for s in range(n_rounds // 2):
            zslab = zt[:, s * SLAB : (s + 1) * SLAB]

            # ---- z^2 fp16 for the whole slab, split across ACT and GPSIMD
            # (frees the critical DVE entirely; GPSIMD TT-mult is walrus-legal
            # and otherwise idle) ----
            zsq = zsqpool.tile([128, SLAB], F16)
            nc.scalar.square(zsq[:, :ZSQ_ACT], zslab[:, :ZSQ_ACT])
            nc.gpsimd.tensor_tensor(
                zsq[:, ZSQ_ACT:], zslab[:, ZSQ_ACT:], zslab[:, ZSQ_ACT:],
                mybir.AluOpType.mult,
            )

            ev = epool.tile([128, SLAB], F32)
            for half in range(2):
                r = 2 * s + half
                zs = zslab[:, half * RFD : (half + 1) * RFD]
                evh = ev[:, half * RFD : (half + 1) * RFD]

                # ---- d2 into PSUM: rows 8v+g ----
                ps = pspool.tile([128, RFD], F32)
                for h in range(RFD // 512):
                    sl = slice(512 * h, 512 * h + 512)
                    nc.tensor.matmul(ps[:, sl], w1_s[:], zs[:, sl], start=True, stop=False)
                for h in range(RFD // 512):
                    sl = slice(512 * h, 512 * h + 512)
                    nc.tensor.matmul(
                        ps[:, sl], wsq_s[:], zsq[:, half * RFD + sl.start : half * RFD + sl.stop],
                        start=False, stop=True,
                    )

                # ---- evacuate to SBUF fused with sqrt: ev = sqrt(d2) ----
                nc.scalar.sqrt(evh, ps[:])

            # ---- pack index into low 4 mantissa bits, whole slab at once ----
            lo = ev[:].bitcast(U16).rearrange("p (n two) -> p n two", two=2)[:, :, 0]
            nc.vector.tensor_scalar(
                lo, lo, -16, vvec_s[:],
                op0=mybir.AluOpType.bitwise_and, op1=mybir.AluOpType.bitwise_or,
            )

            for half in range(2):
                r = 2 * s + half
                evh = ev[:, half * RFD : (half + 1) * RFD]

                # ---- transpose to pixel-major PSUM (bit-exact f32) ----
                tp = tpool.tile([128, RFD], F32)
                for b in range(RFD // 128):
                    sl = slice(128 * b, 128 * b + 128)
                    nc.tensor.transpose(tp[:, sl], evh[:, sl], ident_s[:])

                # ---- segmented strided min-reduce -> packed dmin ----
                osl = slice(r * OPR, r * OPR + OPR)
                vview = tp[:].rearrange("p (b v g) -> p b g v", b=RFD // 128, v=K, g=G)
                nc.vector.tensor_reduce(dm_acc[:, osl], vview, mybir.AxisListType.X, MIN)

            # stream out finished output in two late waves so only the last
            # couple of rounds' sliver remains after the final reduce
            if n_rounds >= 16 and r == n_rounds - 1 - n_rounds // 8:
                cut1 = (r + 1) * OPR
                nc.sync.dma_start(dm_d[:, :cut1], dm_acc[:, :cut1])
            if n_rounds >= 16 and r == n_rounds - 3:
                cut2 = (r + 1) * OPR
                nc.sync.dma_start(dm_d[:, cut1:cut2], dm_acc[:, cut1:cut2])
        mid_out = (n_rounds - 2) * OPR if n_rounds >= 16 else 0
        nc.sync.dma_start(dm_d[:, mid_out:], dm_acc[:, mid_out:])

    nc.compile()
    return nc


def _weights(vertices):
    V = np.asarray(vertices, dtype=np.float32)              # (16, 15)
    vv = (V.astype(np.float64) ** 2).sum(1).astype(np.float32)
    vv_hi = vv.astype(np.float16)
    vv_lo = (vv - vv_hi.astype(np.float32)).astype(np.float16)
    w1 = np.zeros((128, 128), dtype=np.float16)
    wsq = np.zeros((128, 128), dtype=np.float16)
    for g in range(G):
        for v in range(K):
            col = 8 * v + g
            w1[16 * g : 16 * g + 15, col] = (-2.0 * V[v]).astype(np.float16)
            w1[16 * g + 15, col] = vv_hi[v]
            wsq[16 * g : 16 * g + 15, col] = 1.0
            wsq[16 * g + 15, col] = vv_lo[v]
    ident = np.eye(128, dtype=np.float32)
    vvec = (np.arange(128, dtype=np.uint16) >> 3).reshape(128, 1).astype(np.uint16)
    return w1, wsq, ident, vvec


def _prep_zt(z_core):
    # z_core: (15, N_loc) f32 -> [128, GBLK] f16 with ones rows at 16g+15
    n = z_core.shape[1]
    gblk = n // G
    zt = np.empty((128, gblk), dtype=np.float16)
    zg = z_core.reshape(C, G, gblk)
    for g in range(G):
        zt[16 * g : 16 * g + 15] = zg[:, g].astype(np.float16)
        zt[16 * g + 15] = np.float16(1.0)
    return zt


def _unscramble(arr, n_rounds):
    # arr [128, n_rounds*64] laid (q, (r, b, g)) -> per-group-major flat pixels
    a = np.asarray(arr).reshape(128, n_rounds, -1, 8)      # q, r, b, g
    a = a.transpose(3, 1, 2, 0)                            # g, r, b, q
    return np.ascontiguousarray(a).reshape(-1)             # g-major flat


def kernel(z, vertices):
    z = np.ascontiguousarray(np.asarray(z, dtype=np.float32))
    lx, ly = z.shape[1], z.shape[2]
    n = lx * ly
    z_fl = z.reshape(C, n)
    n_loc = n // N_CORES

    if "nc" not in _CACHE:
        _CACHE["nc"] = build_nc()
    nc = _CACHE["nc"]

    w1, wsq, ident, vvec = _weights(vertices)
    in_maps = []
    for c in range(N_CORES):
        zt = _prep_zt(z_fl[:, c * n_loc : (c + 1) * n_loc])
        in_maps.append(
            {"zt": zt, "w1": w1, "wsq": wsq, "ident": ident, "vvec": vvec}
        )
    res = run_bass_kernel_spmd(nc, in_maps, list(range(N_CORES)))

    Xs, Ds = [], []
    for c in range(N_CORES):
        dmc = _unscramble(np.asarray(res.results[c]["dm_o"]), N_ROUNDS)
        Xs.append((dmc.view(np.int32) & 15).astype(np.int32))
        Ds.append(dmc.astype(np.float32))
    X = np.concatenate(Xs).reshape(lx, ly)
    D = np.concatenate(Ds).reshape(lx, ly)
    return X, D


if __name__ == "__main__":
    print("module ok")


# revision 18
# speedup vs baseline: 1.0757x; 1.0170x over previous
"""Trainium2 Bass kernel for vq_codebook argmin (nn_GUMSampler) — v3.

Per pixel p (4M pixels): d2[v] = ||z_p - vertex_v||^2 over 16 vertices in
R^15; outputs argmin index (int32) and min distance (f32).

Pixels sharded 8 ways across cores; per core N=524288 pixels in G=8 groups.

Per-core pipeline (64 rounds of 8192 pixels, FD=1024 per group-round):
  PE    d2 = w1 @ z + wsq @ z^2 into PSUM rows 8v+g   (fp16 in, f32 accum;
        vv split hi/lo over the two ones-row slots for near-f32 accuracy)
  DVE+ACT  z^2 in fp16 (free-dim split across both engines)
  ACT   evacuate PSUM -> SBUF fused with sqrt: ev = sqrt(d2) (monotonic,
        so min/argmin are preserved and no separate sqrt pass is needed)
  DVE   pack candidate index v into the low 4 mantissa bits, touching only
        the LOW uint16 of each f32 (TS 2x_2P on half the elements)
  PE    transpose packed f32 (bit-exact) -> PSUM pixel-major
  DVE   one segmented strided tensor_reduce(min) over the 16 candidates
        (reads PSUM directly; packed min = value + argmin in one word)
The packed f32 min IS the output word: its value is dmin (sqrt is fused
into the PSUM evacuation, low-bit perturbation ~1e-6) and its low 4 bits
are the argmin index, extracted host-side. One output DMA at the end.
"""

import sys

sys.path.insert(0, "/opt/trn_rl_repo")

from contextlib import ExitStack

import numpy as np

import concourse.bacc as bacc
import concourse.tile as tile
from concourse import mybir
from concourse.bass_utils import run_bass_kernel_spmd

F32 = mybir.dt.float32
F16 = mybir.dt.float16
BF16 = mybir.dt.bfloat16
I32 = mybir.dt.int32
I8 = mybir.dt.int8
U16 = mybir.dt.uint16
MIN = mybir.AluOpType.min

K = 16            # vertices
C = 15            # channels
G = 8             # pixel groups (PSUM row = 8v+g)
RFD = 1024        # pixels per group per round
N_CORES = 8
LX = LY = 2048
N_TOTAL = LX * LY
N_LOC = N_TOTAL // N_CORES          # 524288
N_ROUNDS = N_LOC // (G * RFD)       # 64
GBLK = N_LOC // G                   # 65536 pixels per group
ZSQ_ACT = 768                       # zsq slab columns on ACT (rest on GPSIMD)
OPR = RFD // 16                     # output columns per round

_CACHE = {}


def build_nc(n_rounds=N_ROUNDS):
    gblk = n_rounds * RFD
    nc = bacc.Bacc("TRN2", target_bir_lowering=False, debug=False)

    zt_d = nc.dram_tensor("zt", [128, gblk], F16, kind="ExternalInput")
    w1_d = nc.dram_tensor("w1", [128, 128], F16, kind="ExternalInput")
    wsq_d = nc.dram_tensor("wsq", [128, 128], F16, kind="ExternalInput")
    ident_d = nc.dram_tensor("ident", [128, 128], F32, kind="ExternalInput")
    vvec_d = nc.dram_tensor("vvec", [128, 1], U16, kind="ExternalInput")
    dm_d = nc.dram_tensor("dm_o", [128, n_rounds * OPR], F32, kind="ExternalOutput")

    with tile.TileContext(nc) as tc, ExitStack() as ctx:
        cpool = ctx.enter_context(tc.tile_pool(name="consts", bufs=1))
        w1_s = cpool.tile([128, 128], F16)
        wsq_s = cpool.tile([128, 128], F16)
        ident_s = cpool.tile([128, 128], F32)
        vvec_s = cpool.tile([128, 1], U16)
        nc.sync.dma_start(w1_s[:], w1_d[:])
        nc.sync.dma_start(wsq_s[:], wsq_d[:])
        nc.sync.dma_start(ident_s[:], ident_d[:])
        nc.sync.dma_start(vvec_s[:], vvec_d[:])

        ztpool = ctx.enter_context(tc.tile_pool(name="zt", bufs=1))
        zt = ztpool.tile([128, gblk], F16)
        # load z in graded chunks (small first) so compute starts early
        if n_rounds >= 16:
            bounds = [0, 1, 3, 6, 14]
            step = (n_rounds - 14) // 5
            for i in range(1, 5):
                bounds.append(14 + step * i)
            bounds.append(n_rounds)
        else:
            bounds = list(range(n_rounds + 1))
        for ch in range(len(bounds) - 1):
            lo, hi = bounds[ch] * RFD, bounds[ch + 1] * RFD
            if hi > lo:
                nc.sync.dma_start(zt[:, lo:hi], zt_d[:, lo:hi])

        zsqpool = ctx.enter_context(tc.tile_pool(name="zsq", bufs=3))
        pspool = ctx.enter_context(tc.tile_pool(name="d2ps", bufs=2, space="PSUM"))
        epool = ctx.enter_context(tc.tile_pool(name="evac", bufs=4))
        tpool = ctx.enter_context(tc.tile_pool(name="tps", bufs=2, space="PSUM"))
        pmpool = ctx.enter_context(tc.tile_pool(name="pm", bufs=4))
        opool = ctx.enter_context(tc.tile_pool(name="outs", bufs=1))
        dm_acc = opool.tile([128, n_rounds * OPR], F32)

        SLAB = 2 * RFD
        for s in range(n_rounds // 2):
            zslab = zt[:, s * SLAB : (s + 1) * SLAB]

            # ---- z^2 fp16 for the whole slab, split across ACT and GPSIMD
            # (frees the critical DVE entirely; GPSIMD TT-mult is walrus-legal
            # and otherwise idle) ----
            zsq = zsqpool.tile([128, SLAB], F16)
            nc.scalar.square(zsq[:, :ZSQ_ACT], zslab[:, :ZSQ_ACT])
            nc.gpsimd.tensor_tensor(
                zsq[:, ZSQ_ACT:], zslab[:, ZSQ_ACT:], zslab[:, ZSQ_ACT:],
                mybir.AluOpType.mult,
            )

            ev = epool.tile([128, SLAB], F32)
            for half in range(2):
                r = 2 * s + half
                zs = zslab[:, half * RFD : (half + 1) * RFD]
                evh = ev[:, half * RFD : (half + 1) * RFD]

                # ---- d2 into PSUM: rows 8v+g ----
                ps = pspool.tile([128, RFD], F32)
                for h in range(RFD // 512):
                    sl = slice(512 * h, 512 * h + 512)
                    nc.tensor.matmul(ps[:, sl], w1_s[:], zs[:, sl], start=True, stop=False)
                for h in range(RFD // 512):
                    sl = slice(512 * h, 512 * h + 512)
                    nc.tensor.matmul(
                        ps[:, sl], wsq_s[:], zsq[:, half * RFD + sl.start : half * RFD + sl.stop],
                        start=False, stop=True,
                    )

                # ---- evacuate to SBUF fused with sqrt: ev = sqrt(d2) ----
                nc.scalar.sqrt(evh, ps[:])

                # ---- pack index into low 4 mantissa bits (low-u16 lanes);
                # per-half so it never waits on the slab's second evac ----
                lo = evh.bitcast(U16).rearrange("p (n two) -> p n two", two=2)[:, :, 0]
                nc.vector.tensor_scalar(
                    lo, lo, -16, vvec_s[:],
                    op0=mybir.AluOpType.bitwise_and, op1=mybir.AluOpType.bitwise_or,
                )

            for half in range(2):
                r = 2 * s + half
                evh = ev[:, half * RFD : (half + 1) * RFD]

                # ---- transpose to pixel-major PSUM (bit-exact f32) ----
                tp = tpool.tile([128, RFD], F32)
                for b in range(RFD // 128):
                    sl = slice(128 * b, 128 * b + 128)
                    nc.tensor.transpose(tp[:, sl], evh[:, sl], ident_s[:])

                # ---- segmented strided min-reduce -> packed dmin ----
                osl = slice(r * OPR, r * OPR + OPR)
                vview = tp[:].rearrange("p (b v g) -> p b g v", b=RFD // 128, v=K, g=G)
                nc.vector.tensor_reduce(dm_acc[:, osl], vview, mybir.AxisListType.X, MIN)

            # stream out finished output in two late waves so only the last
            # couple of rounds' sliver remains after the final reduce
            if n_rounds >= 16 and r == n_rounds - 1 - n_rounds // 8:
                cut1 = (r + 1) * OPR
                nc.sync.dma_start(dm_d[:, :cut1], dm_acc[:, :cut1])
            if n_rounds >= 16 and r == n_rounds - 3:
                cut2 = (r + 1) * OPR
                nc.sync.dma_start(dm_d[:, cut1:cut2], dm_acc[:, cut1:cut2])
        mid_out = (n_rounds - 2) * OPR if n_rounds >= 16 else 0
        nc.sync.dma_start(dm_d[:, mid_out:], dm_acc[:, mid_out:])

    nc.compile()
    return nc


def _weights(vertices):
    V = np.asarray(vertices, dtype=np.float32)              # (16, 15)
    vv = (V.astype(np.float64) ** 2).sum(1).astype(np.float32)
    vv_hi = vv.astype(np.float16)
    vv_lo = (vv - vv_hi.astype(np.float32)).astype(np.float16)
    w1 = np.zeros((128, 128), dtype=np.float16)
    wsq = np.zeros((128, 128), dtype=np.float16)
    for g in range(G):
        for v in range(K):
            col = 8 * v + g
            w1[16 * g : 16 * g + 15, col] = (-2.0 * V[v]).astype(np.float16)
            w1[16 * g + 15, col] = vv_hi[v]
            wsq[16 * g : 16 * g + 15, col] = 1.0
            wsq[16 * g + 15, col] = vv_lo[v]
    ident = np.eye(128, dtype=np.float32)
    vvec = (np.arange(128, dtype=np.uint16) >> 3).reshape(128, 1).astype(np.uint16)
    return w1, wsq, ident, vvec


def _prep_zt(z_core):
    # z_core: (15, N_loc) f32 -> [128, GBLK] f16 with ones rows at 16g+15
    n = z_core.shape[1]
    gblk = n // G
    zt = np.empty((128, gblk), dtype=np.float16)
    zg = z_core.reshape(C, G, gblk)
    for g in range(G):
        zt[16 * g : 16 * g + 15] = zg[:, g].astype(np.float16)
        zt[16 * g + 15] = np.float16(1.0)
    return zt


def _unscramble(arr, n_rounds):
    # arr [128, n_rounds*64] laid (q, (r, b, g)) -> per-group-major flat pixels
    a = np.asarray(arr).reshape(128, n_rounds, -1, 8)      # q, r, b, g
    a = a.transpose(3, 1, 2, 0)                            # g, r, b, q
    return np.ascontiguousarray(a).reshape(-1)             # g-major flat


def kernel(z, vertices):
    z = np.ascontiguousarray(np.asarray(z, dtype=np.float32))
    lx, ly = z.shape[1], z.shape[2]
    n = lx * ly
    z_fl = z.reshape(C, n)
    n_loc = n // N_CORES

    if "nc" not in _CACHE:
        _CACHE["nc"] = build_nc()
    nc = _CACHE["nc"]

    w1, wsq, ident, vvec = _weights(vertices)
    in_maps = []
    for c in range(N_CORES):
        zt = _prep_zt(z_fl[:, c * n_loc : (c + 1) * n_loc])
        in_maps.append(
            {"zt": zt, "w1": w1, "wsq": wsq, "ident": ident, "vvec": vvec}
        )
    res = run_bass_kernel_spmd(nc, in_maps, list(range(N_CORES)))

    Xs, Ds = [], []
    for c in range(N_CORES):
        dmc = _unscramble(np.asarray(res.results[c]["dm_o"]), N_ROUNDS)
        Xs.append((dmc.view(np.int32) & 15).astype(np.int32))
        Ds.append(dmc.astype(np.float32))
    X = np.concatenate(Xs).reshape(lx, ly)
    D = np.concatenate(Ds).reshape(lx, ly)
    return X, D


if __name__ == "__main__":
    print("module ok")


# revision 20
# speedup vs baseline: 1.0757x; 1.0000x over previous
"""Trainium2 Bass kernel for vq_codebook argmin (nn_GUMSampler) — v3.

Per pixel p (4M pixels): d2[v] = ||z_p - vertex_v||^2 over 16 vertices in
R^15; outputs argmin index (int32) and min distance (f32).

Pixels sharded 8 ways across cores; per core N=524288 pixels in G=8 groups.

Per-core pipeline (64 rounds of 8192 pixels, FD=1024 per group-round):
  PE    d2 = w1 @ z + wsq @ z^2 into PSUM rows 8v+g   (fp16 in, f32 accum;
        vv split hi/lo over the two ones-row slots for near-f32 accuracy)
  DVE+ACT  z^2 in fp16 (free-dim split across both engines)
  ACT   evacuate PSUM -> SBUF fused with sqrt: ev = sqrt(d2) (monotonic,
        so min/argmin are preserved and no separate sqrt pass is needed)
  DVE   pack candidate index v into the low 4 mantissa bits, touching only
        the LOW uint16 of each f32 (TS 2x_2P on half the elements)
  PE    transpose packed f32 (bit-exact) -> PSUM pixel-major
  DVE   one segmented strided tensor_reduce(min) over the 16 candidates
        (reads PSUM directly; packed min = value + argmin in one word)
The packed f32 min IS the output word: its value is dmin (sqrt is fused
into the PSUM evacuation, low-bit perturbation ~1e-6) and its low 4 bits
are the argmin index, extracted host-side. One output DMA at the end.
"""

import sys

sys.path.insert(0, "/opt/trn_rl_repo")

from contextlib import ExitStack

import numpy as np

import concourse.bacc as bacc
import concourse.tile as tile
from concourse import mybir
from concourse.bass_utils import run_bass_kernel_spmd

F32 = mybir.dt.float32
F16 = mybir.dt.float16
BF16 = mybir.dt.bfloat16
I32 = mybir.dt.int32
I8 = mybir.dt.int8
U16 = mybir.dt.uint16
MIN = mybir.AluOpType.min

K = 16            # vertices
C = 15            # channels
G = 8             # pixel groups (PSUM row = 8v+g)
RFD = 1024        # pixels per group per round
N_CORES = 8
LX = LY = 2048
N_TOTAL = LX * LY
N_LOC = N_TOTAL // N_CORES          # 524288
N_ROUNDS = N_LOC // (G * RFD)       # 64
GBLK = N_LOC // G                   # 65536 pixels per group
ZSQ_ACT = 768                       # zsq slab columns on ACT (rest on GPSIMD)
OPR = RFD // 16                     # output columns per round

_CACHE = {}


def build_nc(n_rounds=N_ROUNDS):
    gblk = n_rounds * RFD
    nc = bacc.Bacc("TRN2", target_bir_lowering=False, debug=False)

    zt_d = nc.dram_tensor("zt", [128, gblk], F16, kind="ExternalInput")
    w1_d = nc.dram_tensor("w1", [128, 128], F16, kind="ExternalInput")
    wsq_d = nc.dram_tensor("wsq", [128, 128], F16, kind="ExternalInput")
    ident_d = nc.dram_tensor("ident", [128, 128], F32, kind="ExternalInput")
    vvec_d = nc.dram_tensor("vvec", [128, 1], U16, kind="ExternalInput")
    dm_d = nc.dram_tensor("dm_o", [128, n_rounds * OPR], F32, kind="ExternalOutput")

    with tile.TileContext(nc) as tc, ExitStack() as ctx:
        cpool = ctx.enter_context(tc.tile_pool(name="consts", bufs=1))
        w1_s = cpool.tile([128, 128], F16)
        wsq_s = cpool.tile([128, 128], F16)
        ident_s = cpool.tile([128, 128], F32)
        vvec_s = cpool.tile([128, 1], U16)
        nc.sync.dma_start(w1_s[:], w1_d[:])
        nc.sync.dma_start(wsq_s[:], wsq_d[:])
        nc.sync.dma_start(ident_s[:], ident_d[:])
        nc.sync.dma_start(vvec_s[:], vvec_d[:])

        ztpool = ctx.enter_context(tc.tile_pool(name="zt", bufs=1))
        zt = ztpool.tile([128, gblk], F16)
        # load z in graded chunks (small first) so compute starts early
        if n_rounds >= 16:
            bounds = [0, 1, 3, 6, 14]
            step = (n_rounds - 14) // 5
            for i in range(1, 5):
                bounds.append(14 + step * i)
            bounds.append(n_rounds)
        else:
            bounds = list(range(n_rounds + 1))
        for ch in range(len(bounds) - 1):
            lo, hi = bounds[ch] * RFD, bounds[ch + 1] * RFD
            if hi > lo:
                nc.sync.dma_start(zt[:, lo:hi], zt_d[:, lo:hi])

        zsqpool = ctx.enter_context(tc.tile_pool(name="zsq", bufs=4))
        pspool = ctx.enter_context(tc.tile_pool(name="d2ps", bufs=2, space="PSUM"))
        epool = ctx.enter_context(tc.tile_pool(name="evac", bufs=4))
        tpool = ctx.enter_context(tc.tile_pool(name="tps", bufs=2, space="PSUM"))
        pmpool = ctx.enter_context(tc.tile_pool(name="pm", bufs=4))
        opool = ctx.enter_context(tc.tile_pool(name="outs", bufs=1))
        dm_acc = opool.tile([128, n_rounds * OPR], F32)

        SLAB = 2 * RFD
        for s in range(n_rounds // 2):
            zslab = zt[:, s * SLAB : (s + 1) * SLAB]

            # ---- z^2 fp16 for the whole slab, split across ACT and GPSIMD
            # (frees the critical DVE entirely; GPSIMD TT-mult is walrus-legal
            # and otherwise idle) ----
            zsq = zsqpool.tile([128, SLAB], F16)
            nc.scalar.square(zsq[:, :ZSQ_ACT], zslab[:, :ZSQ_ACT])
            nc.gpsimd.tensor_tensor(
                zsq[:, ZSQ_ACT:], zslab[:, ZSQ_ACT:], zslab[:, ZSQ_ACT:],
                mybir.AluOpType.mult,
            )

            ev = epool.tile([128, SLAB], F32)
            for half in range(2):
                r = 2 * s + half
                zs = zslab[:, half * RFD : (half + 1) * RFD]
                evh = ev[:, half * RFD : (half + 1) * RFD]

                # ---- d2 into PSUM: rows 8v+g ----
                ps = pspool.tile([128, RFD], F32)
                for h in range(RFD // 512):
                    sl = slice(512 * h, 512 * h + 512)
                    nc.tensor.matmul(ps[:, sl], w1_s[:], zs[:, sl], start=True, stop=False)
                for h in range(RFD // 512):
                    sl = slice(512 * h, 512 * h + 512)
                    nc.tensor.matmul(
                        ps[:, sl], wsq_s[:], zsq[:, half * RFD + sl.start : half * RFD + sl.stop],
                        start=False, stop=True,
                    )

                # ---- evacuate to SBUF fused with sqrt: ev = sqrt(d2) ----
                nc.scalar.sqrt(evh, ps[:])

                # ---- pack index into low 4 mantissa bits (low-u16 lanes);
                # per-half so it never waits on the slab's second evac ----
                lo = evh.bitcast(U16).rearrange("p (n two) -> p n two", two=2)[:, :, 0]
                nc.vector.tensor_scalar(
                    lo, lo, -16, vvec_s[:],
                    op0=mybir.AluOpType.bitwise_and, op1=mybir.AluOpType.bitwise_or,
                )

            for half in range(2):
                r = 2 * s + half
                evh = ev[:, half * RFD : (half + 1) * RFD]

                # ---- transpose to pixel-major PSUM (bit-exact f32) ----
                tp = tpool.tile([128, RFD], F32)
                for b in range(RFD // 128):
                    sl = slice(128 * b, 128 * b + 128)
                    nc.tensor.transpose(tp[:, sl], evh[:, sl], ident_s[:])

                # ---- segmented strided min-reduce -> packed dmin ----
                osl = slice(r * OPR, r * OPR + OPR)
                vview = tp[:].rearrange("p (b v g) -> p b g v", b=RFD // 128, v=K, g=G)
                nc.vector.tensor_reduce(dm_acc[:, osl], vview, mybir.AxisListType.X, MIN)

            # stream out finished output in two late waves so only the last
            # couple of rounds' sliver remains after the final reduce
            if n_rounds >= 16 and r == n_rounds - 1 - n_rounds // 8:
                cut1 = (r + 1) * OPR
                nc.sync.dma_start(dm_d[:, :cut1], dm_acc[:, :cut1])
            if n_rounds >= 16 and r == n_rounds - 3:
                cut2 = (r + 1) * OPR
                nc.sync.dma_start(dm_d[:, cut1:cut2], dm_acc[:, cut1:cut2])
        mid_out = (n_rounds - 2) * OPR if n_rounds >= 16 else 0
        nc.sync.dma_start(dm_d[:, mid_out:], dm_acc[:, mid_out:])

    nc.compile()
    return nc


def _weights(vertices):
    V = np.asarray(vertices, dtype=np.float32)              # (16, 15)
    vv = (V.astype(np.float64) ** 2).sum(1).astype(np.float32)
    vv_hi = vv.astype(np.float16)
    vv_lo = (vv - vv_hi.astype(np.float32)).astype(np.float16)
    w1 = np.zeros((128, 128), dtype=np.float16)
    wsq = np.zeros((128, 128), dtype=np.float16)
    for g in range(G):
        for v in range(K):
            col = 8 * v + g
            w1[16 * g : 16 * g + 15, col] = (-2.0 * V[v]).astype(np.float16)
            w1[16 * g + 15, col] = vv_hi[v]
            wsq[16 * g : 16 * g + 15, col] = 1.0
            wsq[16 * g + 15, col] = vv_lo[v]
    ident = np.eye(128, dtype=np.float32)
    vvec = (np.arange(128, dtype=np.uint16) >> 3).reshape(128, 1).astype(np.uint16)
    return w1, wsq, ident, vvec


def _prep_zt(z_core):
    # z_core: (15, N_loc) f32 -> [128, GBLK] f16 with ones rows at 16g+15
    n = z_core.shape[1]
    gblk = n // G
    zt = np.empty((128, gblk), dtype=np.float16)
    zg = z_core.reshape(C, G, gblk)
    for g in range(G):
        zt[16 * g : 16 * g + 15] = zg[:, g].astype(np.float16)
        zt[16 * g + 15] = np.float16(1.0)
    return zt


def _unscramble(arr, n_rounds):
    # arr [128, n_rounds*64] laid (q, (r, b, g)) -> per-group-major flat pixels
    a = np.asarray(arr).reshape(128, n_rounds, -1, 8)      # q, r, b, g
    a = a.transpose(3, 1, 2, 0)                            # g, r, b, q
    return np.ascontiguousarray(a).reshape(-1)             # g-major flat


def kernel(z, vertices):
    z = np.ascontiguousarray(np.asarray(z, dtype=np.float32))
    lx, ly = z.shape[1], z.shape[2]
    n = lx * ly
    z_fl = z.reshape(C, n)
    n_loc = n // N_CORES

    if "nc" not in _CACHE:
        _CACHE["nc"] = build_nc()
    nc = _CACHE["nc"]

    w1, wsq, ident, vvec = _weights(vertices)
    in_maps = []
    for c in range(N_CORES):
        zt = _prep_zt(z_fl[:, c * n_loc : (c + 1) * n_loc])
        in_maps.append(
            {"zt": zt, "w1": w1, "wsq": wsq, "ident": ident, "vvec": vvec}
        )
    res = run_bass_kernel_spmd(nc, in_maps, list(range(N_CORES)))

    Xs, Ds = [], []
    for c in range(N_CORES):
        dmc = _unscramble(np.asarray(res.results[c]["dm_o"]), N_ROUNDS)
        Xs.append((dmc.view(np.int32) & 15).astype(np.int32))
        Ds.append(dmc.astype(np.float32))
    X = np.concatenate(Xs).reshape(lx, ly)
    D = np.concatenate(Ds).reshape(lx, ly)
    return X, D


if __name__ == "__main__":
    print("module ok")
